# revision 6
# baseline (speedup 1.0000x reference)
import sys

sys.path.insert(0, "/opt/trn_rl_repo")

import numpy as np

N_CORES = 8
B, T, C = 2, 2048, 1024
H, D = 16, 64
HPC = H // N_CORES          # heads per core = 2
CPC = HPC * D               # channels per core = 128
TW = (B * T) // N_CORES     # token window per core = 512
NK = C // 128               # k-tiles = 8
NEG = -200.0                # additive mask (exp(scale*NEG) ~ 1.4e-11)

_CACHE = {}
LAST_EXEC_NS = None


def _build():
    import concourse.tile as tile
    from concourse import bacc, mybir

    f32 = mybir.dt.float32
    f32r = mybir.dt.float32r
    Exp = mybir.ActivationFunctionType.Exp

    nc = bacc.Bacc(None, num_devices=N_CORES)

    xT_in = nc.declare_dram_parameter("xT", [C, B * T], f32r, isOutput=False)
    wq_in = nc.declare_dram_parameter("wq", [C, CPC], f32r, isOutput=False)
    wk_in = nc.declare_dram_parameter("wk", [C, CPC], f32r, isOutput=False)
    wv_in = nc.declare_dram_parameter("wv", [C, CPC], f32r, isOutput=False)
    wp_in = nc.declare_dram_parameter("wp", [C, C], f32r, isOutput=False)
    bp_in = nc.declare_dram_parameter("bp", [1, C], f32r, isOutput=False)
    id_in = nc.declare_dram_parameter("ident", [128, 128], f32r, isOutput=False)
    mk_in = nc.declare_dram_parameter("mask", [128, 2048], f32r, isOutput=False)
    on_in = nc.declare_dram_parameter("ones", [1, 128], f32r, isOutput=False)
    oc_in = nc.declare_dram_parameter("onescol", [128, B * 16], f32r, isOutput=False)
    y_out = nc.declare_dram_parameter("y", [TW, C], f32, isOutput=True)

    with tile.TileContext(nc) as tc:
        with tc.tile_pool(name="ps", bufs=1, space="PSUM") as ps, \
             tc.tile_pool(name="dram", bufs=1, space="DRAM") as dram, \
             tc.tile_pool(name="sb", bufs=1) as sb:

            # ---- persistent SBUF tiles ----
            qT = sb.tile([128, B * T], f32r, name="qT")
            kT = sb.tile([128, B * T], f32r, name="kT")
            vT = sb.tile([128, B * T], f32r, name="vT")
            v_nat = sb.tile([128, B * 16, 2 * (D + 1)], f32r, name="v_nat")
            attnT = sb.tile([128, B * T], f32r, name="attnT")
            ident = sb.tile([128, 128], f32r, name="ident")
            maskM = sb.tile([128, 4, 512], f32r, name="maskM")
            ones = sb.tile([1, 128], f32r, name="ones")
            bias_sb = sb.tile([1, C], f32r, name="bias_sb")

            # host-precomputed constants: identity (transposes + mask-add
            # matmuls), additive causal masks, ones rows, v_nat ones columns
            nc.sync.dma_start(out=ident, in_=id_in[:])
            nc.sync.dma_start(out=maskM, in_=mk_in[:])
            nc.sync.dma_start(out=ones, in_=on_in[:])
            nc.sync.dma_start(out=v_nat[:, :, D:D + 1], in_=oc_in[:])
            nc.sync.dma_start(out=v_nat[:, :, 2 * D + 1:2 * D + 2], in_=oc_in[:])
            nc.sync.dma_start(out=bias_sb, in_=bp_in[:])

            # ================= qkv phase =================
            with tc.tile_pool(name="qkv", bufs=1) as sbq:
                wq_sb = sbq.tile([128, NK, CPC], f32r, name="wq_sb")
                wk_sb = sbq.tile([128, NK, CPC], f32r, name="wk_sb")
                wv_sb = sbq.tile([128, NK, CPC], f32r, name="wv_sb")
                for w_sb, w_in in ((wq_sb, wq_in), (wk_sb, wk_in), (wv_sb, wv_in)):
                    for k in range(NK):
                        nc.sync.dma_start(
                            out=w_sb[:, k, :], in_=w_in[128 * k:128 * (k + 1), :])

                for b in range(B):
                    for tch in range(4):
                        col = b * T + 512 * tch
                        xt = sbq.tile([128, NK, 512], f32r, tag="xt", bufs=2)
                        for k in range(NK):
                            nc.sync.dma_start(
                                out=xt[:, k, :],
                                in_=xT_in[128 * k:128 * (k + 1), col:col + 512])
                        for w_sb, dstT in ((wq_sb, qT), (wk_sb, kT), (wv_sb, vT)):
                            acc = ps.tile([128, 512], f32, tag="sm", bufs=2)
                            for k in range(NK):
                                nc.tensor.matmul(acc, w_sb[:, k, :], xt[:, k, :],
                                                 start=(k == 0), stop=(k == NK - 1))
                            nc.vector.tensor_copy(out=dstT[:, col:col + 512], in_=acc)
                    # transpose v into natural layout for this batch
                    for kb in range(16):
                        tr = ps.tile([128, 128], f32r, tag="sm", bufs=2)
                        nc.tensor.transpose(tr, vT[:, b * T + 128 * kb:b * T + 128 * (kb + 1)], ident)
                        nc.vector.tensor_copy(out=v_nat[:, 16 * b + kb, 0:D], in_=tr[:, 0:D])
                        nc.vector.tensor_copy(out=v_nat[:, 16 * b + kb, D + 1:2 * D + 1], in_=tr[:, D:2 * D])

            # ================= attention + proj =================
            with tc.tile_pool(name="proj", bufs=1) as sbp:
                wp_sb = sbp.tile([128, NK, C], f32r, name="wp_sb")
                a2a_sb = sbp.tile([128, NK, TW], f32r, name="a2a_sb")
                y_sb = sbp.tile([128, 4, C], f32, name="y_sb")
                for k in range(NK):
                    nc.sync.dma_start(out=wp_sb[:, k, :], in_=wp_in[128 * k:128 * (k + 1), :])

                send_d = dram.tile([N_CORES * CPC, TW], f32r, name="send_d")
                recv_d = dram.tile([N_CORES * CPC, TW], f32r, name="recv_d")

                for b in range(B):
                    for j in range(4):          # 512-query chunks
                        qcol = b * T + 512 * j
                        for hl in range(HPC):   # local head
                            hr = D * hl
                            av = ps.tile([128, 512], f32, tag="av", bufs=2)
                            for g in range(j + 1):   # groups of 4 key-blocks
                                sp = ps.tile([128, 2048], f32, tag="sp", bufs=1)
                                for i in range(4):
                                    kb = 4 * g + i
                                    nc.tensor.matmul(
                                        sp[:, 512 * i:512 * (i + 1)],
                                        kT[hr:hr + D, b * T + 128 * kb:b * T + 128 * (kb + 1)],
                                        qT[hr:hr + D, qcol:qcol + 512],
                                        start=True, stop=(g != j))
                                if g == j:
                                    for i in range(4):
                                        nc.tensor.matmul(
                                            sp[:, 512 * i:512 * (i + 1)],
                                            ident, maskM[:, i, :],
                                            start=False, stop=True)
                                P = sb.tile([128, 2048], f32r, tag="p", bufs=2)
                                nc.scalar.activation(out=P, in_=sp, func=Exp, scale=0.125)
                                for i in range(4):
                                    kb = 4 * g + i
                                    nc.tensor.matmul(
                                        av[0:D + 1, :],
                                        v_nat[:, 16 * b + kb, (D + 1) * hl:(D + 1) * (hl + 1)],
                                        P[:, 512 * i:512 * (i + 1)],
                                        start=(g == 0 and i == 0),
                                        stop=(g == j and i == 3))
                            rec = sb.tile([1, 512], f32r, tag="rec", bufs=2)
                            with nc.allow_low_precision(reason="float32r is bit-identical to float32"):
                                nc.vector.reciprocal(out=rec, in_=av[D:D + 1, :])
                            bc = ps.tile([D, 512], f32, tag="sm", bufs=2)
                            nc.tensor.matmul(bc, ones[0:1, 0:D], rec, start=True, stop=True)
                            bcs = sb.tile([D, 512], f32, tag="bcs", bufs=2)
                            nc.vector.tensor_copy(out=bcs, in_=bc)
                            nc.vector.tensor_tensor(
                                out=attnT[hr:hr + D, qcol:qcol + 512],
                                in0=av[0:D, :], in1=bcs, op=mybir.AluOpType.mult)
                        w = 4 * b + j
                        nc.gpsimd.dma_start(
                            out=send_d[CPC * w:CPC * (w + 1), :],
                            in_=attnT[:, 512 * w:512 * (w + 1)])

                nc.gpsimd.collective_compute(
                    "AllToAll",
                    mybir.AluOpType.bypass,
                    replica_groups=[list(range(N_CORES))],
                    ins=[send_d.opt()],
                    outs=[recv_d.opt()],
                )

                for k in range(NK):
                    nc.sync.dma_start(out=a2a_sb[:, k, :], in_=recv_d[128 * k:128 * (k + 1), :])
                for tb in range(4):
                    for cc in range(2):
                        yp = ps.tile([128, 512], f32, tag="sm", bufs=2)
                        for k in range(NK):
                            nc.tensor.matmul(
                                yp, a2a_sb[:, k, 128 * tb:128 * (tb + 1)],
                                wp_sb[:, k, 512 * cc:512 * (cc + 1)],
                                start=(k == 0), stop=False)
                        nc.tensor.matmul(
                            yp, ones, bias_sb[0:1, 512 * cc:512 * (cc + 1)],
                            start=False, stop=True)
                        nc.vector.tensor_copy(out=y_sb[:, tb, 512 * cc:512 * (cc + 1)], in_=yp)
                    nc.gpsimd.dma_start(out=y_out[128 * tb:128 * (tb + 1), :], in_=y_sb[:, tb, :])

    nc.finalize()
    return nc


def kernel(x, Wq, Wk, Wv, Wproj, bproj):
    global LAST_EXEC_NS
    from concourse.bass_utils import run_bass_kernel_spmd

    if "nc" not in _CACHE:
        _CACHE["nc"] = _build()
    nc = _CACHE["nc"]

    xT = np.ascontiguousarray(x.reshape(B * T, C).T.astype(np.float32))
    wp = np.ascontiguousarray(Wproj.astype(np.float32))
    bp = np.ascontiguousarray(bproj.reshape(1, C).astype(np.float32))
    ident = np.eye(128, dtype=np.float32)
    pi = np.arange(128)[:, None]
    qi = np.arange(512)[None, :]
    mask = np.stack(
        [np.where(qi - pi - 128 * i >= 0, 0.0, NEG) for i in range(4)],
        axis=1).astype(np.float32).reshape(128, 2048)
    onesr = np.ones((1, 128), dtype=np.float32)
    onescol = np.ones((128, B * 16), dtype=np.float32)
    in_maps = []
    for c in range(N_CORES):
        in_maps.append({
            "xT": xT,
            "wq": np.ascontiguousarray(np.concatenate([Wq[2 * c], Wq[2 * c + 1]], axis=1)),
            "wk": np.ascontiguousarray(np.concatenate([Wk[2 * c], Wk[2 * c + 1]], axis=1)),
            "wv": np.ascontiguousarray(np.concatenate([Wv[2 * c], Wv[2 * c + 1]], axis=1)),
            "wp": wp,
            "bp": bp,
            "ident": ident,
            "mask": mask,
            "ones": onesr,
            "onescol": onescol,
        })

    res = run_bass_kernel_spmd(nc, in_maps, list(range(N_CORES)))
    LAST_EXEC_NS = res.exec_time_ns
    y = np.concatenate([res.results[c]["y"] for c in range(N_CORES)], axis=0)
    return np.ascontiguousarray(y.reshape(B, T, C).astype(np.float32))
